# revision 2
# baseline (speedup 1.0000x reference)
"""Trainium2 Bass kernel for nn_GCNNMultiKernel (gnn_message_passing).

Sharding: 8 cores = 4 graphs x 2 node-column halves. Core c owns graph
b = c // 2 and node columns [half*1024, half*1024+1024), half = c % 2.

Optimized engine split (vs. the all-ACT-exp baseline):
  - d2 is built once on PE (PSUM), then stored to SBUF by ACT as
    v = relu(c - d2)  (f16), where c = 126/(smax*log2e) is a shared clamp.
    All consumers of d2 are re-expressed in terms of v so the clamp is free.
  - The 24 per-(layer,band,edge) kernel exps are split across two engines:
      DVE route: K in bf16 via the integer-bits exp trick ("Schraudolph"):
        i16 = round(schA_le * v + schB_le), bit-pattern IS the bf16 kernel
        value (t always lands in [120, 16249] so no saturation edge cases;
        underflow was pre-clamped via c).  tensor_scalar f16->i16 runs at
        DVE 2x (0.52 ns/elem) vs ACT's 0.83 ns/elem.
      ACT route: K = exp(s*v - s*c) straight to fp8e4 (exact e4m3 RTN cast),
        feeding fp8 DoubleRow stage-1 matmuls (2 k-tiles per pass, 0.5
        cy/row = 4x bf16 throughput).
  - Stage-1 lhsT carries [ (emb - mu)/32 | ones/64 ] so PSUM rows are
    (adj - mu*deg)/32 and deg/64: the spatialnorm mean-fold costs nothing,
    a_t = (32*inv) * ps1 is a single ACT copy with per-partition scale, and
    all tensors stay inside fp8/bf16 range.  deg/64 and the /32 folds are
    compensated host-side in the stage-2 weights (x64 deg ops; x32 layer-0
    adj ops) and on-device via inv32 = 32*inv.
  - a_t + epilogue (lin copy, nlin relu) run on ACT (same act-table set as
    Exp: no table reloads); d_t on DVE; spatialnorm applications on Pool
    (SBUF-only engine).  Stage-2 stays bf16.
  - Between layers: pair AllGather of the un-normalized local emb via DRAM,
    bn_stats/aggr + Newton rsqrt on DVE, then Pool applies (x-mu)/32 to the
    gathered quarters before PE re-transposes them into the next layer's
    lhsT (bf16 + fp8 copies).
  - After layer 3: per-core column-sum of final local emb -> output (32,).

Host: builds per-core inputs (slices, -2*coord, r2 layouts, folded/stacked
weights, the sigma-derived table [exp scale/bias, schraudolph A/B, clamp])
and computes the tiny final head in numpy.

_build_program(repeat=K) wraps the whole compute body in a device For_i
loop; used only for timing (wall-clock difference between K and 1 reps).
"""
import sys
from contextlib import ExitStack

sys.path.insert(0, "/opt/trn_rl_repo")

import numpy as np
import ml_dtypes
import concourse.bass as bass
import concourse.bacc as bacc
import concourse.tile as tile
from concourse import mybir
from concourse.bass_utils import run_bass_kernel_spmd

F32 = mybir.dt.float32
F16 = mybir.dt.float16
BF16 = mybir.dt.bfloat16
FP8 = mybir.dt.float8e4
I16 = mybir.dt.int16
AF = mybir.ActivationFunctionType
ALU = mybir.AluOpType
PM = mybir.MatmulPerfMode

B = 4
N = 2048
C = 1024          # owned columns per core
P = 128           # SBUF partitions per m-tile
MT = N // P       # 16 m-tiles
NBW = 512         # band width (one PSUM bank of fp32)
NB = C // NBW     # 2 bands
E = 4             # edge kernels
L = 3             # layers
F0 = 16           # input feature dim (layer 0)
F = 32            # node feature dim
G = 16            # m-tiles per exp instruction
EPS_SN = 1e-8
EPS_IN = 1e-5

LOG2E = 1.4426950408889634
SCH_B0 = 16248.1          # schraudolph base constant (mean-centered)
LE = L * E

_CACHE = {}


def _dve_route(lay, nb, e):
    """True -> kernel exp on DVE (bf16 trick); False -> ACT (fp8 + DR)."""
    if lay == 0:
        return e in (0, 1, 2)                      # 6 of 8 on DVE
    return (e in (0, 2)) or (e == 1 and nb == 0)   # 5 of 8 on DVE


def _build_program(repeat=1, fake_cc=False):
    nc = bacc.Bacc("TRN2", target_bir_lowering=False, debug=False, num_devices=8)

    # ------------- DRAM I/O (per-core data supplied via in_maps) -------------
    d_c2 = nc.dram_tensor("c2_all", (4, N), BF16, kind="ExternalInput")
    d_cm = nc.dram_tensor("coords_my", (4, C), BF16, kind="ExternalInput")
    # tabs columns: [0:LE] exp scale (+s), [LE:2LE] exp bias (-s*c),
    # [2LE:3LE] schA (+128*log2e*s), [3LE:4LE] schB, [4LE] clamp c
    d_tabs = nc.dram_tensor("tabs", (1, 4 * LE + 1), F32, kind="ExternalInput")
    d_embT0 = nc.dram_tensor("embT0_ones", (N, 2 * F), BF16, kind="ExternalInput")
    d_embT0_8 = nc.dram_tensor("embT0_8", (N, 2 * F), FP8, kind="ExternalInput")
    d_emb0 = nc.dram_tensor("emb0_loc", (F0, C), BF16, kind="ExternalInput")
    d_wT0 = nc.dram_tensor("wT0", (F0, 9, 2 * F), BF16, kind="ExternalInput")
    d_wT12 = nc.dram_tensor("wT12", (F, 2, 9, 2 * F), BF16, kind="ExternalInput")
    d_bcat = nc.dram_tensor("bcat", (1, L, 2 * F), BF16, kind="ExternalInput")
    d_ident = nc.dram_tensor("ident", (F, F), F32, kind="ExternalInput")

    d_pp = nc.dram_tensor("pooled_partial", (F, 1), F32, kind="ExternalOutput")

    # collective bounce buffers: one pair per (boundary, band)
    cc_in = [[nc.dram_tensor(f"cc_in{i}_{j}", (F, NBW), F32) for j in range(NB)]
             for i in range(L - 1)]
    cc_out = [[nc.dram_tensor(f"cc_out{i}_{j}", (2, F, NBW), F32) for j in range(NB)]
              for i in range(L - 1)]
    groups = [[0, 1], [2, 3], [4, 5], [6, 7]]

    with tile.TileContext(nc) as tc, ExitStack() as ctx:
        consts = ctx.enter_context(tc.tile_pool(name="consts", bufs=1))
        d2pool = ctx.enter_context(tc.tile_pool(name="d2", bufs=1))
        kpool_d = ctx.enter_context(tc.tile_pool(name="ktd", bufs=3))
        kpool_8 = ctx.enter_context(tc.tile_pool(name="kt8", bufs=3))
        adpool = ctx.enter_context(tc.tile_pool(name="spread", bufs=2))
        embpool = ctx.enter_context(tc.tile_pool(name="emb", bufs=1))
        small = ctx.enter_context(tc.tile_pool(name="small", bufs=4))
        ps_misc = ctx.enter_context(tc.tile_pool(name="ps_misc", bufs=2, space="PSUM"))
        ps_s1 = ctx.enter_context(tc.tile_pool(name="ps_s1", bufs=4, space="PSUM"))
        ps_s2 = ctx.enter_context(tc.tile_pool(name="ps_s2", bufs=2, space="PSUM"))

        # ------------------------- constants to SBUF -------------------------
        c2_sb = consts.tile([4, N], BF16)
        nc.sync.dma_start(out=c2_sb, in_=d_c2.ap())
        cm_sb = consts.tile([4, C], BF16)
        nc.sync.dma_start(out=cm_sb, in_=d_cm.ap())
        ones_n = consts.tile([1, NBW], BF16)
        nc.vector.memset(ones_n, 1.0)
        ident_sb = consts.tile([F, F], F32)
        nc.sync.dma_start(out=ident_sb, in_=d_ident.ap())
        bcat_sb = consts.tile([1, L, 2 * F], BF16)
        nc.sync.dma_start(out=bcat_sb, in_=d_bcat.ap())
        wT0_sb = consts.tile([F0, 9, 2 * F], BF16)
        nc.sync.dma_start(out=wT0_sb, in_=d_wT0.ap())
        wT12_sb = consts.tile([F, 2, 9, 2 * F], BF16)
        nc.sync.dma_start(out=wT12_sb, in_=d_wT12.ap())
        # sigma-derived tables broadcast across partitions: (128, 4*LE+1)
        tabs_sb = consts.tile([P, 4 * LE + 1], F32)
        tabs_bcast = bass.AP(tensor=d_tabs.ap().tensor, offset=0,
                             ap=[[0, P], [1, 4 * LE + 1]])
        nc.sync.dma_start(out=tabs_sb, in_=tabs_bcast)
        # layer-0 lhsT: [embT0 | 0 | ones/64 | 0] rearranged (mt p) f -> p mt f
        embT0_sb = consts.tile([P, MT, 2 * F], BF16)
        nc.sync.dma_start(
            out=embT0_sb,
            in_=d_embT0.ap().rearrange("(mt p) f -> p mt f", p=P),
        )
        embT0_8sb = consts.tile([P, MT, 2 * F], FP8)
        nc.sync.dma_start(
            out=embT0_8sb,
            in_=d_embT0_8.ap().rearrange("(mt p) f -> p mt f", p=P),
        )
        emb0_sb = consts.tile([F0, C], BF16)
        nc.sync.dma_start(out=emb0_sb, in_=d_emb0.ap())

        # lhsT for layers 1,2: per-mtile bf16 tiles (DVE route) and per-pair
        # fp8 tiles (ACT route / DoubleRow).  ones-block = 1/64 (deg fold).
        embT_tiles = []
        for _mt in range(MT):
            _t = embpool.tile([P, 2 * F], BF16, tag=f"embT{_mt}")
            nc.vector.memset(_t[:, F:2 * F], 1.0 / 64.0)
            embT_tiles.append(_t)
        embT8_pairs = []
        for _g in range(MT // 2):
            _t = embpool.tile([P, 2, 2 * F], FP8, tag=f"embT8_{_g}")
            nc.vector.memset(_t[:, :, F:2 * F], 1.0 / 64.0)
            embT8_pairs.append(_t)

        # persistent emb tensors
        emb_loc = embpool.tile([F, C], BF16)       # normalized local (layers 1,2)
        new_emb = embpool.tile([F, C], F32)        # layer output (local cols)
        # gathered (un-normalized) emb quarters: q = rank*NB + nb
        emb_q = []
        for _q in range(4):
            _t = embpool.tile([F, NBW], F32, tag=f"emb_q{_q}")
            emb_q.append(_t)

        warm = consts.tile([1, 1], F32)
        nc.vector.memset(warm, 0.0)
        nc.scalar.activation(warm, warm, AF.Exp, bias=0.0, scale=1.0)

        # stage-1 issue orders: quarters fed by the band-0 gather first
        TORDER = [0, 1, 2, 3, 8, 9, 10, 11, 4, 5, 6, 7, 12, 13, 14, 15]
        PORDER = [0, 1, 4, 5, 2, 3, 6, 7]          # mt pairs (2g, 2g+1)

        c_ap = tabs_sb[:, 4 * LE:4 * LE + 1]

        def _emit_body():
            # ------------------- phase 1: d2 -> v = relu(c - d2) -------------
            d2_sb = d2pool.tile([P, MT, C], F16)
            for nb in range(NB):
                for mt in range(MT):
                    ps = ps_misc.tile([P, NBW], F32, tag="misc")
                    nc.tensor.matmul(
                        ps,
                        c2_sb[:, mt * P:(mt + 1) * P],
                        cm_sb[:, nb * NBW:(nb + 1) * NBW],
                        start=True, stop=True)
                    nc.scalar.activation(
                        d2_sb[:, mt, nb * NBW:(nb + 1) * NBW], ps,
                        AF.Relu, bias=c_ap, scale=-1.0)

            # -------------------------- phase 2: layers -------------------------
            for lay in range(L):
                fin = F0 if lay == 0 else F
                ecur = emb0_sb if lay == 0 else emb_loc
                pp_bands = None
                if lay == L - 1:
                    pp_bands = []
                    for _j in range(NB):
                        ppt = small.tile([F, 1], F32,
                                         tag=f"ppb{_j}", name=f"ppb{_j}")
                        pp_bands.append(ppt)
                for nb in range(NB):
                    ncols = slice(nb * NBW, (nb + 1) * NBW)
                    a_t = adpool.tile([F, E, NBW], BF16, tag="a_t")
                    d_t = adpool.tile([F, E, NBW], BF16, tag="d_t")
                    for e in range(E):
                        le = lay * E + e
                        ps1 = ps_s1.tile([2 * F, NBW], F32)
                        if _dve_route(lay, nb, e):
                            # DVE: i16 = rint(schA*v + schB); bits are bf16 K
                            ktd = kpool_d.tile([P, G, NBW], I16, tag="ktd")
                            if lay == 0 and nb == 0 and e == 0:
                                # halve the first op: start after 8 d2 m-tiles
                                nc.vector.tensor_scalar(
                                    ktd[:, 0:G // 2, :],
                                    d2_sb[:, 0:G // 2, ncols],
                                    tabs_sb[:, 2 * LE + le:2 * LE + le + 1],
                                    tabs_sb[:, 3 * LE + le:3 * LE + le + 1],
                                    op0=ALU.mult, op1=ALU.add)
                                nc.vector.tensor_scalar(
                                    ktd[:, G // 2:G, :],
                                    d2_sb[:, G // 2:G, ncols],
                                    tabs_sb[:, 2 * LE + le:2 * LE + le + 1],
                                    tabs_sb[:, 3 * LE + le:3 * LE + le + 1],
                                    op0=ALU.mult, op1=ALU.add)
                            else:
                                nc.vector.tensor_scalar(
                                    ktd, d2_sb[:, :, ncols],
                                    tabs_sb[:, 2 * LE + le:2 * LE + le + 1],
                                    tabs_sb[:, 3 * LE + le:3 * LE + le + 1],
                                    op0=ALU.mult, op1=ALU.add)
                            for mi, mt in enumerate(TORDER):
                                lhsT_mt = (embT0_sb[:, mt, :] if lay == 0
                                           else embT_tiles[mt])
                                nc.tensor.matmul(
                                    ps1,
                                    lhsT_mt,
                                    ktd[:, mt, :].bitcast(BF16),
                                    start=(mi == 0), stop=(mi == MT - 1))
                        else:
                            # ACT: K = exp(s*v - s*c) -> fp8e4; DoubleRow MMs
                            kt8 = kpool_8.tile([P, G, NBW], FP8, tag="kt8")
                            nc.scalar.activation(
                                kt8, d2_sb[:, :, ncols], AF.Exp,
                                bias=tabs_sb[:, LE + le:LE + le + 1],
                                scale=tabs_sb[:, le:le + 1])
                            for gi, g in enumerate(PORDER):
                                lhsT_g = (embT0_8sb[:, 2 * g:2 * g + 2, :]
                                          if lay == 0 else embT8_pairs[g])
                                nc.tensor.matmul(
                                    ps1,
                                    lhsT_g,
                                    kt8[:, 2 * g:2 * g + 2, :],
                                    start=(gi == 0), stop=(gi == MT // 2 - 1),
                                    perf_mode=PM.DoubleRow)
                        # a_t on ACT: per-partition scaled PSUM->SBUF copy
                        if lay == 0:
                            nc.scalar.activation(
                                a_t[0:fin, e, :], ps1[0:fin, :],
                                AF.Copy, bias=0.0, scale=1.0 / 32.0)
                        else:
                            nc.scalar.activation(
                                a_t[0:fin, e, :], ps1[0:fin, :],
                                AF.Copy, bias=0.0, scale=inv32[0:fin, 0:1])
                        nc.vector.tensor_mul(
                            d_t[0:fin, e, :], ps1[F:F + fin, :], ecur[:, ncols])
                    # stage 2: 9 op-terms + bias accumulate into one PSUM bank
                    ps2 = ps_s2.tile([2 * F, NBW], F32)

                    def wt(op):
                        if lay == 0:
                            return wT0_sb[:, op, :]
                        return wT12_sb[:, lay - 1, op, :]

                    nc.tensor.matmul(ps2, wt(0), ecur[:, ncols],
                                     start=True, stop=False)
                    for e in range(E):
                        nc.tensor.matmul(ps2, wt(1 + e), d_t[0:fin, e, :],
                                         start=False, stop=False)
                    for e in range(E):
                        nc.tensor.matmul(ps2, wt(5 + e), a_t[0:fin, e, :],
                                         start=False, stop=False)
                    nc.tensor.matmul(ps2, bcat_sb[0:1, lay, :], ones_n,
                                     start=False, stop=True)
                    # epilogue on ACT: copy lin half; ReLU nlin half via
                    # partition-32 scratch + DMA shift
                    nc.scalar.activation(new_emb[0:F // 2, ncols],
                                         ps2[0:F // 2, :],
                                         AF.Copy, bias=0.0, scale=1.0)
                    nlin_tmp = adpool.tile([F + F // 2, NBW], F32,
                                           tag="nlin_tmp")
                    nc.scalar.activation(nlin_tmp[F:F + F // 2, :],
                                         ps2[F:F + F // 2, :],
                                         AF.Relu, bias=0.0, scale=1.0)
                    nc.sync.dma_start(out=new_emb[F // 2:F, ncols],
                                      in_=nlin_tmp[F:F + F // 2, :])
                    if lay == L - 1:
                        ppn = pp_bands[nb]
                        nc.vector.reduce_sum(ppn, new_emb[:, ncols],
                                             axis=mybir.AxisListType.X)

                    if lay < L - 1:
                        # gather this band across the pair right away
                        nc.sync.dma_start(out=cc_in[lay][nb].ap(),
                                          in_=new_emb[:, ncols])
                        if fake_cc:
                            nc.sync.dma_start(out=cc_out[lay][nb][0],
                                              in_=cc_in[lay][nb].ap())
                            nc.sync.dma_start(out=cc_out[lay][nb][1],
                                              in_=cc_in[lay][nb].ap())
                        else:
                            nc.gpsimd.collective_compute(
                                "AllGather", ALU.bypass, replica_groups=groups,
                                ins=[cc_in[lay][nb].ap().opt()],
                                outs=[cc_out[lay][nb].ap().opt()])
                        nc.sync.dma_start(out=emb_q[nb],
                                          in_=cc_out[lay][nb][0])
                        nc.sync.dma_start(out=emb_q[NB + nb],
                                          in_=cc_out[lay][nb][1])

                if lay < L - 1:
                    # spatialnorm stats over full N per feature row
                    stats = small.tile([F, 4, 6], F32)
                    for sg in range(4):
                        nc.vector.bn_stats(out=stats[:, sg, :],
                                           in_=emb_q[sg])
                    mv = small.tile([F, 2], F32)
                    nc.vector.bn_aggr(out=mv, in_=stats)
                    varu = small.tile([F, 1], F32)
                    nc.vector.tensor_scalar_mul(varu, mv[:, 1:2],
                                                float(N) / (N - 1))
                    # rsqrt on DVE only (keeps ACT on the exp table set):
                    # bit-trick seed + 3 Newton iterations, then sd = v*y
                    yr = small.tile([F, 1], F32)
                    iv = varu.bitcast(mybir.dt.int32)
                    nc.vector.tensor_scalar(yr.bitcast(mybir.dt.int32), iv,
                                            1, None, op0=ALU.logical_shift_right)
                    nc.vector.tensor_scalar(yr.bitcast(mybir.dt.int32),
                                            yr.bitcast(mybir.dt.int32),
                                            0xFFFFFFFF, None,
                                            op0=ALU.bitwise_xor)
                    nc.vector.tensor_scalar(yr.bitcast(mybir.dt.int32),
                                            yr.bitcast(mybir.dt.int32),
                                            0x5f3759df + 1, None,
                                            op0=ALU.add)
                    t_a = small.tile([F, 1], F32)
                    for _ in range(3):
                        nc.vector.tensor_mul(t_a, yr, yr)
                        nc.vector.tensor_mul(t_a, t_a, varu)
                        nc.vector.tensor_scalar(t_a, t_a, -0.5, 1.5,
                                                op0=ALU.mult, op1=ALU.add)
                        nc.vector.tensor_mul(yr, yr, t_a)
                    sd = small.tile([F, 1], F32)
                    nc.vector.tensor_mul(sd, varu, yr)
                    den = small.tile([F, 1], F32)
                    nc.vector.tensor_scalar_add(den, sd, EPS_SN)
                    inv = small.tile([F, 1], F32, tag="inv")
                    nc.vector.reciprocal(inv, den)
                    inv32 = small.tile([F, 1], F32, tag="inv32")
                    nc.vector.tensor_scalar_mul(inv32, inv, 32.0)
                    # Pool: quarters -> (x - mu)/32 in place, then PE
                    # transposes build the next layer's lhsT (bf16 + fp8)
                    for q in range(4):
                        nc.gpsimd.tensor_scalar(
                            emb_q[q], emb_q[q], mv[:, 0:1], 1.0 / 32.0,
                            op0=ALU.subtract, op1=ALU.mult)
                    for mt in TORDER:
                        q = mt // (MT // 4)
                        lo = (mt % (MT // 4)) * P
                        pst_full = ps_misc.tile([P, NBW], F32, tag="misc")
                        pst = pst_full[:, 0:F]
                        nc.tensor.transpose(
                            pst, emb_q[q][:, lo:lo + P], ident_sb)
                        nc.vector.tensor_copy(embT_tiles[mt][:, 0:F], pst)
                        nc.vector.tensor_copy(
                            embT8_pairs[mt // 2][:, mt % 2, 0:F], pst)
                    # normalized local emb for stage 2 of the next layer
                    nc.gpsimd.tensor_scalar(emb_loc, new_emb, mv[:, 0:1],
                                            inv[:, 0:1],
                                            op0=ALU.subtract, op1=ALU.mult)
                else:
                    pp = small.tile([F, 1], F32)
                    nc.vector.tensor_add(pp, pp_bands[0], pp_bands[1])
                    nc.sync.dma_start(out=d_pp.ap(), in_=pp)

        if repeat > 1:
            with tc.For_i(0, repeat, 1):
                _emit_body()
        else:
            _emit_body()

    nc.compile()
    return nc


def _host_inputs(global_input, sigmas, w_lin0, b_lin0, w_nlin0, b_nlin0,
                 w_lin, b_lin, w_nlin, b_nlin):
    gi = np.asarray(global_input, np.float32)
    sig = np.asarray(sigmas, np.float32)
    svals = (1.0 / (sig.reshape(-1) ** 2)).astype(np.float64)   # (LE,)
    smax = float(svals.max())
    c = 126.0 / (smax * LOG2E)
    tabs = np.zeros((1, 4 * LE + 1), np.float32)
    tabs[0, 0:LE] = svals                       # exp scale (+s)
    tabs[0, LE:2 * LE] = -svals * c             # exp bias (-s*c)
    tabs[0, 2 * LE:3 * LE] = 128.0 * LOG2E * svals
    tabs[0, 3 * LE:4 * LE] = SCH_B0 - 128.0 * LOG2E * svals * c
    tabs[0, 4 * LE] = c

    # stage-2 rows at 0:16 (lin) and 32:48 (nlin); weight folds: deg ops x64
    # (ones-block = 1/64), layer-0 adj ops x32 (a_t = ps1/32)
    def fold(op, lay):
        if 1 <= op <= 4:
            return 64.0
        if op >= 5 and lay == 0:
            return 32.0
        return 1.0

    wl0 = np.asarray(w_lin0, np.float32)
    wn0 = np.asarray(w_nlin0, np.float32)
    wT0 = np.zeros((F0, 9, 2 * F), np.float32)
    for op in range(9):
        s = fold(op, 0)
        wT0[:, op, 0:16] = s * wl0[:, op * F0:(op + 1) * F0].T
        wT0[:, op, F:F + 16] = s * wn0[:, op * F0:(op + 1) * F0].T
    wT12 = np.zeros((F, 2, 9, 2 * F), np.float32)
    for l in range(2):
        wl = np.asarray(w_lin[l], np.float32)
        wn = np.asarray(w_nlin[l], np.float32)
        for op in range(9):
            s = fold(op, l + 1)
            wT12[:, l, op, 0:16] = s * wl[:, op * F:(op + 1) * F].T
            wT12[:, l, op, F:F + 16] = s * wn[:, op * F:(op + 1) * F].T
    bcat = np.zeros((1, L, 2 * F), np.float32)
    bl = [np.asarray(b_lin0, np.float32), np.asarray(b_lin[0], np.float32),
          np.asarray(b_lin[1], np.float32)]
    bn = [np.asarray(b_nlin0, np.float32), np.asarray(b_nlin[0], np.float32),
          np.asarray(b_nlin[1], np.float32)]
    for l in range(L):
        bcat[0, l, 0:16] = bl[l]
        bcat[0, l, F:F + 16] = bn[l]
    ident = np.eye(F, dtype=np.float32)

    in_maps = []
    for cid in range(8):
        b = cid // 2
        half = cid % 2
        cols = slice(half * C, half * C + C)
        coord = gi[b, :2, :]                      # (2, 2048)
        r2 = (coord ** 2).sum(axis=0)             # (2048,)
        c2r2 = np.empty((4, N), np.float32)       # lhsT rows [-2x, -2y, 1, r2]
        c2r2[0:2] = -2.0 * coord
        c2r2[2] = 1.0
        c2r2[3] = r2
        crm = np.empty((4, C), np.float32)        # rhs rows [x, y, r2, 1]
        crm[0:2] = coord[:, cols]
        crm[2] = r2[cols]
        crm[3] = 1.0
        embT0 = np.zeros((N, 2 * F), np.float32)
        embT0[:, 0:F0] = gi[b].T
        embT0[:, F:F + F0] = 1.0 / 64.0
        in_maps.append(dict(
            c2_all=c2r2.astype(ml_dtypes.bfloat16),
            coords_my=crm.astype(ml_dtypes.bfloat16),
            tabs=tabs,
            embT0_ones=embT0.astype(ml_dtypes.bfloat16),
            embT0_8=embT0.astype(ml_dtypes.float8_e4m3),
            emb0_loc=np.ascontiguousarray(gi[b][:, cols]).astype(ml_dtypes.bfloat16),
            wT0=wT0.astype(ml_dtypes.bfloat16),
            wT12=wT12.astype(ml_dtypes.bfloat16),
            bcat=np.ascontiguousarray(bcat).astype(ml_dtypes.bfloat16),
            ident=ident,
        ))
    return in_maps


def kernel(global_input, sigmas, w_lin0, b_lin0, w_nlin0, b_nlin0,
           w_lin, b_lin, w_nlin, b_nlin, fcl_w, fcl_b):
    if "nc" not in _CACHE:
        _CACHE["nc"] = _build_program()
    nc = _CACHE["nc"]
    in_maps = _host_inputs(global_input, sigmas, w_lin0, b_lin0, w_nlin0,
                           b_nlin0, w_lin, b_lin, w_nlin, b_nlin)
    res = run_bass_kernel_spmd(nc, in_maps, core_ids=list(range(8)))
    pooled = np.empty((B, F), np.float64)
    for b in range(B):
        s0 = res.results[2 * b]["pooled_partial"].reshape(F)
        s1 = res.results[2 * b + 1]["pooled_partial"].reshape(F)
        pooled[b] = (s0.astype(np.float64) + s1.astype(np.float64)) / N
    mu = pooled.mean(axis=1, keepdims=True)
    var = pooled.var(axis=1, keepdims=True)
    normed = (pooled - mu) / np.sqrt(var + EPS_IN)
    logits = normed @ np.asarray(fcl_w, np.float64).T + np.asarray(fcl_b, np.float64)
    out = 1.0 / (1.0 + np.exp(-logits[:, 0]))
    return out.astype(np.float32)


# revision 18
# speedup vs baseline: 1.2680x; 1.2680x over previous
"""Trainium2 Bass kernel for nn_GCNNMultiKernel (gnn_message_passing).

Sharding: 8 cores = 4 graphs x 2 node-column halves. Core c owns graph
b = c // 2 and node columns [half*1024, half*1024+1024), half = c % 2.

Optimized engine split (vs. the all-ACT-exp baseline):
  - d2 is built once on PE (PSUM), then stored to SBUF by ACT as
    v = relu(c - d2)  (f16), where c = 126/(smax*log2e) is a shared clamp.
    All consumers of d2 are re-expressed in terms of v so the clamp is free.
  - The 24 per-(layer,band,edge) kernel exps are split across two engines:
      DVE route: K in bf16 via the integer-bits exp trick ("Schraudolph"):
        i16 = round(schA_le * v + schB_le), bit-pattern IS the bf16 kernel
        value (t always lands in [120, 16249] so no saturation edge cases;
        underflow was pre-clamped via c).  tensor_scalar f16->i16 runs at
        DVE 2x (0.52 ns/elem) vs ACT's 0.83 ns/elem.
      ACT route: K = exp(s*v - s*c) straight to fp8e4 (exact e4m3 RTN cast),
        feeding fp8 DoubleRow stage-1 matmuls (2 k-tiles per pass, 0.5
        cy/row = 4x bf16 throughput).
  - Stage-1 lhsT carries [ emb/32 | ones/64 ] (uncentered!) so PSUM rows are
    adj/32 and deg/64, all inside fp8/bf16 range.  The spatialnorm mean is
    folded at consume time: a_t = inv32*ps1[adj] - (64*inv*mu)*ps1[deg]
    (dsc on ACT with per-partition scale + one DVE stt).  deg/64 and the
    /32 are compensated host-side in the stage-2 weights (x64 deg ops; x32
    layer-0 adj ops).
  - Engine streams are emitted software-pipelined: all-independent exp ops
    lead each engine's stream (LA-tile lookahead + 2 exps hoisted across
    each layer boundary) so in-order engines never head-of-line block on
    PSUM consumers.
  - epilogue + dsc + layer-0 a_t + d2 copies run on ACT (exp/copy/relu all
    live in one act table: no reloads); d_t + stt on DVE; emb_loc on Pool.
    Stage-2 stays bf16.
  - Between layers: pair AllGather of the un-normalized local emb via DRAM;
    per-quarter (gated only by that quarter's gather): bn_stats + PE
    transposes into next-layer lhsT, the /32 riding on the PSUM->SBUF
    copies (bf16 + fp8).  Then bn_aggr + Newton rsqrt -> inv tiles.
  - After layer 3: per-core column-sum of final local emb -> output (32,).

Host: builds per-core inputs (slices, -2*coord, r2 layouts, folded/stacked
weights, the sigma-derived table [exp scale/bias, schraudolph A/B, clamp])
and computes the tiny final head in numpy.

_build_program(repeat=K) wraps the whole compute body in a device For_i
loop; used only for timing (wall-clock difference between K and 1 reps).
"""
import sys
from contextlib import ExitStack

sys.path.insert(0, "/opt/trn_rl_repo")

import numpy as np
import ml_dtypes
import concourse.bass as bass
import concourse.bacc as bacc
import concourse.tile as tile
from concourse import mybir
from concourse.bass_utils import run_bass_kernel_spmd

F32 = mybir.dt.float32
F16 = mybir.dt.float16
BF16 = mybir.dt.bfloat16
FP8 = mybir.dt.float8e4
I16 = mybir.dt.int16
AF = mybir.ActivationFunctionType
ALU = mybir.AluOpType
PM = mybir.MatmulPerfMode

B = 4
N = 2048
C = 1024          # owned columns per core
P = 128           # SBUF partitions per m-tile
MT = N // P       # 16 m-tiles
NBW = 512         # band width (one PSUM bank of fp32)
NB = C // NBW     # 2 bands
E = 4             # edge kernels
L = 3             # layers
F0 = 16           # input feature dim (layer 0)
F = 32            # node feature dim
G = 16            # m-tiles per exp instruction
EPS_SN = 1e-8
EPS_IN = 1e-5

LOG2E = 1.4426950408889634
SCH_B0 = 16248.1          # schraudolph base constant (mean-centered)
LE = L * E

_CACHE = {}

# engine-assignment config (bisectable)
CFG = dict(
    pool_boundary=True,   # emb_q scale + emb_loc on Pool (else DVE)
    at_on_act=True,       # a_t copies on ACT (else DVE)
    epi_on_act=True,      # stage-2 epilogue on ACT (else DVE)
    d2_on_act=True,       # d2 relu-copies on ACT (else DVE)
    use_dr=True,          # fp8 DoubleRow stage-1 on ACT route (else bf16)
    dve_l0=(0, 1, 2),     # edges on DVE route, layer 0
    dve_l12=(0, 2),       # edges on DVE route, layers 1-2 (both bands)
    dve_l12_b0=(1,),      # extra edges on DVE route, band 0 only
)


def _dve_route(lay, nb, e):
    """True -> kernel exp on DVE (bf16 trick); False -> ACT (fp8 + DR)."""
    if lay == 0:
        return e in CFG["dve_l0"]
    return (e in CFG["dve_l12"]) or (e in CFG["dve_l12_b0"] and nb == 0)


def _build_program(repeat=1, fake_cc=False):
    nc = bacc.Bacc("TRN2", target_bir_lowering=False, debug=False, num_devices=8)

    # ------------- DRAM I/O (per-core data supplied via in_maps) -------------
    d_c2 = nc.dram_tensor("c2_all", (4, N), BF16, kind="ExternalInput")
    d_cm = nc.dram_tensor("coords_my", (4, C), BF16, kind="ExternalInput")
    # tabs columns: [0:LE] exp scale (+s), [LE:2LE] exp bias (-s*c),
    # [2LE:3LE] schA (+128*log2e*s), [3LE:4LE] schB, [4LE] clamp c
    d_tabs = nc.dram_tensor("tabs", (1, 4 * LE + 1), F32, kind="ExternalInput")
    d_embT0 = nc.dram_tensor("embT0_ones", (N, 2 * F), BF16, kind="ExternalInput")
    d_embT0_8 = nc.dram_tensor("embT0_8", (N, 2 * F), FP8, kind="ExternalInput")
    d_emb0 = nc.dram_tensor("emb0_loc", (F0, C), BF16, kind="ExternalInput")
    d_wT0 = nc.dram_tensor("wT0", (F0, 9, 2 * F), BF16, kind="ExternalInput")
    d_wT12 = nc.dram_tensor("wT12", (F, 2, 9, 2 * F), BF16, kind="ExternalInput")
    d_bcat = nc.dram_tensor("bcat", (1, L, 2 * F), BF16, kind="ExternalInput")
    d_ident = nc.dram_tensor("ident", (F, F), F32, kind="ExternalInput")

    d_pp = nc.dram_tensor("pooled_partial", (F, 1), F32, kind="ExternalOutput")

    # collective bounce buffers: one pair per (boundary, band)
    cc_in = [[nc.dram_tensor(f"cc_in{i}_{j}", (F, NBW), F32) for j in range(NB)]
             for i in range(L - 1)]
    cc_out = [[nc.dram_tensor(f"cc_out{i}_{j}", (2, F, NBW), F32) for j in range(NB)]
              for i in range(L - 1)]
    groups = [[0, 1], [2, 3], [4, 5], [6, 7]]

    with tile.TileContext(nc) as tc, ExitStack() as ctx:
        consts = ctx.enter_context(tc.tile_pool(name="consts", bufs=1))
        d2pool = ctx.enter_context(tc.tile_pool(name="d2", bufs=1))
        kpool_d = ctx.enter_context(tc.tile_pool(name="ktd", bufs=4))
        kpool_8 = ctx.enter_context(tc.tile_pool(name="kt8", bufs=4))
        adpool = ctx.enter_context(tc.tile_pool(name="spread", bufs=2))
        embpool = ctx.enter_context(tc.tile_pool(name="emb", bufs=1))
        small = ctx.enter_context(tc.tile_pool(name="small", bufs=4))
        ps_misc = ctx.enter_context(tc.tile_pool(name="ps_misc", bufs=2, space="PSUM"))
        ps_s1 = ctx.enter_context(tc.tile_pool(name="ps_s1", bufs=4, space="PSUM"))
        ps_s2 = ctx.enter_context(tc.tile_pool(name="ps_s2", bufs=2, space="PSUM"))

        # ------------------------- constants to SBUF -------------------------
        c2_sb = consts.tile([4, N], BF16)
        nc.sync.dma_start(out=c2_sb, in_=d_c2.ap())
        cm_sb = consts.tile([4, C], BF16)
        nc.sync.dma_start(out=cm_sb, in_=d_cm.ap())
        ones_n = consts.tile([1, NBW], BF16)
        nc.vector.memset(ones_n, 1.0)
        ident_sb = consts.tile([F, F], F32)
        nc.sync.dma_start(out=ident_sb, in_=d_ident.ap())
        bcat_sb = consts.tile([1, L, 2 * F], BF16)
        nc.sync.dma_start(out=bcat_sb, in_=d_bcat.ap())
        wT0_sb = consts.tile([F0, 9, 2 * F], BF16)
        nc.sync.dma_start(out=wT0_sb, in_=d_wT0.ap())
        wT12_sb = consts.tile([F, 2, 9, 2 * F], BF16)
        nc.sync.dma_start(out=wT12_sb, in_=d_wT12.ap())
        # sigma-derived tables broadcast across partitions: (128, 4*LE+1)
        tabs_sb = consts.tile([P, 4 * LE + 1], F32)
        tabs_bcast = bass.AP(tensor=d_tabs.ap().tensor, offset=0,
                             ap=[[0, P], [1, 4 * LE + 1]])
        nc.sync.dma_start(out=tabs_sb, in_=tabs_bcast)
        # layer-0 lhsT: [embT0 | 0 | ones/64 | 0] rearranged (mt p) f -> p mt f
        embT0_sb = consts.tile([P, MT, 2 * F], BF16)
        nc.sync.dma_start(
            out=embT0_sb,
            in_=d_embT0.ap().rearrange("(mt p) f -> p mt f", p=P),
        )
        embT0_8sb = consts.tile([P, MT, 2 * F], FP8)
        nc.sync.dma_start(
            out=embT0_8sb,
            in_=d_embT0_8.ap().rearrange("(mt p) f -> p mt f", p=P),
        )
        emb0_sb = consts.tile([F0, C], BF16)
        nc.sync.dma_start(out=emb0_sb, in_=d_emb0.ap())

        # lhsT for layers 1,2: per-mtile bf16 tiles (DVE route) and per-pair
        # fp8 tiles (ACT route / DoubleRow).  ones-block = 1/64 (deg fold).
        embT_tiles = []
        for _mt in range(MT):
            _t = embpool.tile([P, 2 * F], BF16, tag=f"embT{_mt}")
            nc.vector.memset(_t[:, F:2 * F], 1.0 / 64.0)
            embT_tiles.append(_t)
        embT8_pairs = []
        for _g in range(MT // 2):
            _t = embpool.tile([P, 2, 2 * F], FP8, tag=f"embT8_{_g}")
            nc.vector.memset(_t[:, :, F:2 * F], 1.0 / 64.0)
            embT8_pairs.append(_t)

        # persistent emb tensors
        emb_loc = embpool.tile([F, C], BF16)       # normalized local (layers 1,2)
        new_emb = embpool.tile([F, C], F32)        # layer output (local cols)
        # gathered (un-normalized) emb quarters: q = rank*NB + nb
        emb_q = []
        for _q in range(4):
            _t = embpool.tile([F, NBW], F32, tag=f"emb_q{_q}")
            emb_q.append(_t)

        warm = consts.tile([1, 1], F32)
        nc.vector.memset(warm, 0.0)
        nc.scalar.activation(warm, warm, AF.Exp, bias=0.0, scale=1.0)

        # stage-1 issue orders: quarters fed by the band-0 gather first
        TORDER = [0, 1, 2, 3, 8, 9, 10, 11, 4, 5, 6, 7, 12, 13, 14, 15]
        PORDER = [0, 1, 4, 5, 2, 3, 6, 7]          # mt pairs (2g, 2g+1)

        c_ap = tabs_sb[:, 4 * LE:4 * LE + 1]

        def _emit_body():
            # ------------------- phase 1: d2 -> v = relu(c - d2) -------------
            d2_sb = d2pool.tile([P, MT, C], F16)
            for nb in range(NB):
                for mt in range(MT):
                    ps = ps_misc.tile([P, NBW], F32, tag="misc")
                    nc.tensor.matmul(
                        ps,
                        c2_sb[:, mt * P:(mt + 1) * P],
                        cm_sb[:, nb * NBW:(nb + 1) * NBW],
                        start=True, stop=True)
                    if CFG["d2_on_act"]:
                        nc.scalar.activation(
                            d2_sb[:, mt, nb * NBW:(nb + 1) * NBW], ps,
                            AF.Relu, bias=c_ap, scale=-1.0)
                    else:
                        # timing-bisect only: stores min(d2-c,0) = v-c, same
                        # cost and value-range as v but shifted kernel values
                        nc.vector.tensor_scalar(
                            d2_sb[:, mt, nb * NBW:(nb + 1) * NBW], ps,
                            c_ap, 0.0, op0=ALU.subtract, op1=ALU.min)

            # -------------------------- phase 2: layers -------------------------
            # software-pipelined emission: each engine's instruction stream
            # lists independent exp work ahead of PSUM consumers so in-order
            # engines never head-of-line block on another engine's chain.
            LA = 3                      # consume lookahead (ps_s1 ring - 1)
            HOIST = 2                   # next-layer exps hoisted over boundary
            kt_map = {}                 # (lay, nb, e) -> kt tile
            ps_map = {}                 # (lay, nb, e) -> ps1 tile
            ad_map = {}                 # (lay, nb) -> (a_t, d_t)
            pp_bands = []

            def emit_exp(lay, nb, e):
                le = lay * E + e
                ncols = slice(nb * NBW, (nb + 1) * NBW)
                if _dve_route(lay, nb, e):
                    ktd = kpool_d.tile([P, G, NBW], I16, tag="ktd",
                                       name=f"ktd{lay}{nb}{e}")
                    if lay == 0 and nb == 0 and e == 0:
                        # halve the first op: start after 8 d2 m-tiles
                        for h in range(2):
                            nc.vector.tensor_scalar(
                                ktd[:, h * G // 2:(h + 1) * G // 2, :],
                                d2_sb[:, h * G // 2:(h + 1) * G // 2, ncols],
                                tabs_sb[:, 2 * LE + le:2 * LE + le + 1],
                                tabs_sb[:, 3 * LE + le:3 * LE + le + 1],
                                op0=ALU.mult, op1=ALU.add)
                    else:
                        nc.vector.tensor_scalar(
                            ktd, d2_sb[:, :, ncols],
                            tabs_sb[:, 2 * LE + le:2 * LE + le + 1],
                            tabs_sb[:, 3 * LE + le:3 * LE + le + 1],
                            op0=ALU.mult, op1=ALU.add)
                    kt_map[(lay, nb, e)] = ktd
                elif CFG["use_dr"]:
                    kt8 = kpool_8.tile([P, G, NBW], FP8, tag="kt8",
                                       name=f"kt8{lay}{nb}{e}")
                    nc.scalar.activation(
                        kt8, d2_sb[:, :, ncols], AF.Exp,
                        bias=tabs_sb[:, LE + le:LE + le + 1],
                        scale=tabs_sb[:, le:le + 1])
                    kt_map[(lay, nb, e)] = kt8
                else:
                    ktb = kpool_8.tile([P, G, NBW], BF16, tag="ktb",
                                       name=f"ktb{lay}{nb}{e}")
                    nc.scalar.activation(
                        ktb, d2_sb[:, :, ncols], AF.Exp,
                        bias=tabs_sb[:, LE + le:LE + le + 1],
                        scale=tabs_sb[:, le:le + 1])
                    kt_map[(lay, nb, e)] = ktb

            def emit_mms(lay, nb, e):
                kt = kt_map[(lay, nb, e)]
                ps1 = ps_s1.tile([2 * F, NBW], F32)
                ps_map[(lay, nb, e)] = ps1
                if _dve_route(lay, nb, e):
                    for mi, mt in enumerate(TORDER):
                        lhsT_mt = (embT0_sb[:, mt, :] if lay == 0
                                   else embT_tiles[mt])
                        nc.tensor.matmul(
                            ps1, lhsT_mt, kt[:, mt, :].bitcast(BF16),
                            start=(mi == 0), stop=(mi == MT - 1))
                elif CFG["use_dr"]:
                    for gi, g in enumerate(PORDER):
                        lhsT_g = (embT0_8sb[:, 2 * g:2 * g + 2, :]
                                  if lay == 0 else embT8_pairs[g])
                        nc.tensor.matmul(
                            ps1, lhsT_g, kt[:, 2 * g:2 * g + 2, :],
                            start=(gi == 0), stop=(gi == MT // 2 - 1),
                            perf_mode=PM.DoubleRow)
                else:
                    for mi, mt in enumerate(TORDER):
                        lhsT_mt = (embT0_sb[:, mt, :] if lay == 0
                                   else embT_tiles[mt])
                        nc.tensor.matmul(
                            ps1, lhsT_mt, kt[:, mt, :],
                            start=(mi == 0), stop=(mi == MT - 1))

            def emit_consume(lay, nb, e):
                fin = F0 if lay == 0 else F
                ecur = emb0_sb if lay == 0 else emb_loc
                ncols = slice(nb * NBW, (nb + 1) * NBW)
                if (lay, nb) not in ad_map:
                    a_t = adpool.tile([F, E, NBW], BF16, tag="a_t",
                                      name=f"a_t{lay}{nb}")
                    d_t = adpool.tile([F, E, NBW], BF16, tag="d_t",
                                      name=f"d_t{lay}{nb}")
                    ad_map[(lay, nb)] = (a_t, d_t)
                a_t, d_t = ad_map[(lay, nb)]
                ps1 = ps_map.pop((lay, nb, e))
                # a_t: normalized adjacency from uncentered stage-1 rows:
                # a_t = inv32*ps1[adj] - invmu64*ps1[deg] (mu-fold at consume)
                if lay == 0:
                    if CFG["at_on_act"]:
                        nc.scalar.activation(
                            a_t[0:fin, e, :], ps1[0:fin, :],
                            AF.Copy, bias=0.0, scale=1.0 / 32.0)
                    else:
                        nc.vector.tensor_scalar_mul(
                            a_t[0:fin, e, :], ps1[0:fin, :], 1.0 / 32.0)
                else:
                    dsc = adpool.tile([F, NBW], F32, tag="dsc")
                    if CFG["at_on_act"]:
                        nc.scalar.activation(
                            dsc[0:fin, :], ps1[F:F + fin, :],
                            AF.Copy, bias=0.0, scale=invmu64[0:fin, 0:1])
                    else:
                        nc.vector.tensor_scalar_mul(
                            dsc[0:fin, :], ps1[F:F + fin, :],
                            invmu64[0:fin, 0:1])
                    nc.vector.scalar_tensor_tensor(
                        a_t[0:fin, e, :], ps1[0:fin, :],
                        inv32[0:fin, 0:1], dsc[0:fin, :],
                        op0=ALU.mult, op1=ALU.subtract)
                nc.vector.tensor_mul(
                    d_t[0:fin, e, :], ps1[F:F + fin, :], ecur[:, ncols])
                if e == E - 1:
                    emit_stage2(lay, nb)

            def emit_stage2(lay, nb):
                fin = F0 if lay == 0 else F
                ecur = emb0_sb if lay == 0 else emb_loc
                ncols = slice(nb * NBW, (nb + 1) * NBW)
                a_t, d_t = ad_map.pop((lay, nb))
                ps2 = ps_s2.tile([2 * F, NBW], F32)

                def wt(op):
                    if lay == 0:
                        return wT0_sb[:, op, :]
                    return wT12_sb[:, lay - 1, op, :]

                nc.tensor.matmul(ps2, wt(0), ecur[:, ncols],
                                 start=True, stop=False)
                for e in range(E):
                    nc.tensor.matmul(ps2, wt(1 + e), d_t[0:fin, e, :],
                                     start=False, stop=False)
                for e in range(E):
                    nc.tensor.matmul(ps2, wt(5 + e), a_t[0:fin, e, :],
                                     start=False, stop=False)
                nc.tensor.matmul(ps2, bcat_sb[0:1, lay, :], ones_n,
                                 start=False, stop=True)
                # epilogue: copy lin half; ReLU nlin half via partition-32
                # scratch + DMA shift
                nlin_tmp = adpool.tile([F + F // 2, NBW], F32,
                                       tag="nlin_tmp")
                if CFG["epi_on_act"]:
                    nc.scalar.activation(new_emb[0:F // 2, ncols],
                                         ps2[0:F // 2, :],
                                         AF.Copy, bias=0.0, scale=1.0)
                    nc.scalar.activation(nlin_tmp[F:F + F // 2, :],
                                         ps2[F:F + F // 2, :],
                                         AF.Relu, bias=0.0, scale=1.0)
                else:
                    nc.vector.tensor_copy(new_emb[0:F // 2, ncols],
                                          ps2[0:F // 2, :])
                    nc.vector.tensor_scalar_max(nlin_tmp[F:F + F // 2, :],
                                                ps2[F:F + F // 2, :], 0.0)
                nc.sync.dma_start(out=new_emb[F // 2:F, ncols],
                                  in_=nlin_tmp[F:F + F // 2, :])
                if lay == L - 1:
                    ppt = small.tile([F, 1], F32,
                                     tag=f"ppb{nb}", name=f"ppb{nb}")
                    nc.vector.reduce_sum(ppt, new_emb[:, ncols],
                                         axis=mybir.AxisListType.X)
                    pp_bands.append(ppt)
                else:
                    # gather this band across the pair right away
                    nc.sync.dma_start(out=cc_in[lay][nb].ap(),
                                      in_=new_emb[:, ncols])
                    if fake_cc:
                        nc.sync.dma_start(out=cc_out[lay][nb][0],
                                          in_=cc_in[lay][nb].ap())
                        nc.sync.dma_start(out=cc_out[lay][nb][1],
                                          in_=cc_in[lay][nb].ap())
                    else:
                        nc.gpsimd.collective_compute(
                            "AllGather", ALU.bypass, replica_groups=groups,
                            ins=[cc_in[lay][nb].ap().opt()],
                            outs=[cc_out[lay][nb].ap().opt()])
                    nc.sync.dma_start(out=emb_q[nb],
                                      in_=cc_out[lay][nb][0])
                    nc.sync.dma_start(out=emb_q[NB + nb],
                                      in_=cc_out[lay][nb][1])

            TILES = [(nb, e) for nb in range(NB) for e in range(E)]

            for lay in range(L):
                del pp_bands[:]
                # exps hoisted before the previous boundary block
                done_exp = HOIST if lay > 0 else 0
                queue = []
                for i, (nb, e) in enumerate(TILES):
                    if i >= done_exp:
                        emit_exp(lay, nb, e)
                    emit_mms(lay, nb, e)
                    queue.append((nb, e))
                    if len(queue) > LA:
                        cnb, ce = queue.pop(0)
                        emit_consume(lay, cnb, ce)
                for cnb, ce in queue:
                    emit_consume(lay, cnb, ce)

                if lay < L - 1:
                    # per-quarter work gated only by that quarter's gather:
                    # bn_stats + PE transposes into next-layer lhsT tiles
                    # (the /32 lhsT scale rides on the PSUM->SBUF copies).
                    # band-0 quarters (0, 2) landed mid-layer: do them
                    # first, then hoist next-layer exps to cover the
                    # band-1 gather latency.
                    stats = small.tile([F, 4, 6], F32)

                    def quarter_block(q):
                        nc.vector.bn_stats(out=stats[:, q, :], in_=emb_q[q])
                        for k in range(MT // 4):
                            mt = q * (MT // 4) + k
                            pst_full = ps_misc.tile([P, NBW], F32, tag="misc")
                            pst = pst_full[:, 0:F]
                            nc.tensor.transpose(
                                pst, emb_q[q][:, k * P:(k + 1) * P], ident_sb)
                            nc.vector.tensor_scalar_mul(
                                embT_tiles[mt][:, 0:F], pst, 1.0 / 32.0)
                            nc.vector.tensor_scalar_mul(
                                embT8_pairs[mt // 2][:, mt % 2, 0:F], pst,
                                1.0 / 32.0)

                    quarter_block(0)
                    quarter_block(2)
                    for hb, he in TILES[:HOIST]:
                        emit_exp(lay + 1, hb, he)
                    quarter_block(1)
                    quarter_block(3)
                    mv = small.tile([F, 2], F32)
                    nc.vector.bn_aggr(out=mv, in_=stats)
                    varu = small.tile([F, 1], F32)
                    nc.vector.tensor_scalar_mul(varu, mv[:, 1:2],
                                                float(N) / (N - 1))
                    # rsqrt on DVE only (keeps ACT on the exp table set):
                    # bit-trick seed + 3 Newton iterations, then sd = v*y
                    yr = small.tile([F, 1], F32)
                    iv = varu.bitcast(mybir.dt.int32)
                    nc.vector.tensor_scalar(yr.bitcast(mybir.dt.int32), iv,
                                            1, None, op0=ALU.logical_shift_right)
                    nc.vector.tensor_scalar(yr.bitcast(mybir.dt.int32),
                                            yr.bitcast(mybir.dt.int32),
                                            0xFFFFFFFF, None,
                                            op0=ALU.bitwise_xor)
                    nc.vector.tensor_scalar(yr.bitcast(mybir.dt.int32),
                                            yr.bitcast(mybir.dt.int32),
                                            0x5f3759df + 1, None,
                                            op0=ALU.add)
                    t_a = small.tile([F, 1], F32)
                    for _ in range(3):
                        nc.vector.tensor_mul(t_a, yr, yr)
                        nc.vector.tensor_mul(t_a, t_a, varu)
                        nc.vector.tensor_scalar(t_a, t_a, -0.5, 1.5,
                                                op0=ALU.mult, op1=ALU.add)
                        nc.vector.tensor_mul(yr, yr, t_a)
                    sd = small.tile([F, 1], F32)
                    nc.vector.tensor_mul(sd, varu, yr)
                    den = small.tile([F, 1], F32)
                    nc.vector.tensor_scalar_add(den, sd, EPS_SN)
                    inv = small.tile([F, 1], F32, tag="inv")
                    nc.vector.reciprocal(inv, den)
                    inv32 = small.tile([F, 1], F32, tag="inv32")
                    nc.vector.tensor_scalar_mul(inv32, inv, 32.0)
                    invmu64 = small.tile([F, 1], F32, tag="invmu64")
                    nc.vector.tensor_mul(invmu64, inv, mv[:, 0:1])
                    nc.vector.tensor_scalar_mul(invmu64, invmu64, 64.0)
                    # normalized local emb for stage 2 of the next layer
                    qeng = nc.gpsimd if CFG["pool_boundary"] else nc.vector
                    qeng.tensor_scalar(emb_loc, new_emb, mv[:, 0:1],
                                       inv[:, 0:1],
                                       op0=ALU.subtract, op1=ALU.mult)
                else:
                    pp = small.tile([F, 1], F32)
                    nc.vector.tensor_add(pp, pp_bands[0], pp_bands[1])
                    nc.sync.dma_start(out=d_pp.ap(), in_=pp)

        if repeat > 1:
            with tc.For_i(0, repeat, 1):
                _emit_body()
        else:
            _emit_body()

    nc.compile()
    return nc


def _host_inputs(global_input, sigmas, w_lin0, b_lin0, w_nlin0, b_nlin0,
                 w_lin, b_lin, w_nlin, b_nlin):
    gi = np.asarray(global_input, np.float32)
    sig = np.asarray(sigmas, np.float32)
    svals = (1.0 / (sig.reshape(-1) ** 2)).astype(np.float64)   # (LE,)
    smax = float(svals.max())
    c = 126.0 / (smax * LOG2E)
    tabs = np.zeros((1, 4 * LE + 1), np.float32)
    tabs[0, 0:LE] = svals                       # exp scale (+s)
    tabs[0, LE:2 * LE] = -svals * c             # exp bias (-s*c)
    tabs[0, 2 * LE:3 * LE] = 128.0 * LOG2E * svals
    tabs[0, 3 * LE:4 * LE] = SCH_B0 - 128.0 * LOG2E * svals * c
    tabs[0, 4 * LE] = c

    # stage-2 rows at 0:16 (lin) and 32:48 (nlin); weight folds: deg ops x64
    # (ones-block = 1/64), layer-0 adj ops x32 (a_t = ps1/32)
    def fold(op, lay):
        if 1 <= op <= 4:
            return 64.0
        if op >= 5 and lay == 0:
            return 32.0
        return 1.0

    wl0 = np.asarray(w_lin0, np.float32)
    wn0 = np.asarray(w_nlin0, np.float32)
    wT0 = np.zeros((F0, 9, 2 * F), np.float32)
    for op in range(9):
        s = fold(op, 0)
        wT0[:, op, 0:16] = s * wl0[:, op * F0:(op + 1) * F0].T
        wT0[:, op, F:F + 16] = s * wn0[:, op * F0:(op + 1) * F0].T
    wT12 = np.zeros((F, 2, 9, 2 * F), np.float32)
    for l in range(2):
        wl = np.asarray(w_lin[l], np.float32)
        wn = np.asarray(w_nlin[l], np.float32)
        for op in range(9):
            s = fold(op, l + 1)
            wT12[:, l, op, 0:16] = s * wl[:, op * F:(op + 1) * F].T
            wT12[:, l, op, F:F + 16] = s * wn[:, op * F:(op + 1) * F].T
    bcat = np.zeros((1, L, 2 * F), np.float32)
    bl = [np.asarray(b_lin0, np.float32), np.asarray(b_lin[0], np.float32),
          np.asarray(b_lin[1], np.float32)]
    bn = [np.asarray(b_nlin0, np.float32), np.asarray(b_nlin[0], np.float32),
          np.asarray(b_nlin[1], np.float32)]
    for l in range(L):
        bcat[0, l, 0:16] = bl[l]
        bcat[0, l, F:F + 16] = bn[l]
    ident = np.eye(F, dtype=np.float32)

    in_maps = []
    for cid in range(8):
        b = cid // 2
        half = cid % 2
        cols = slice(half * C, half * C + C)
        coord = gi[b, :2, :]                      # (2, 2048)
        r2 = (coord ** 2).sum(axis=0)             # (2048,)
        c2r2 = np.empty((4, N), np.float32)       # lhsT rows [-2x, -2y, 1, r2]
        c2r2[0:2] = -2.0 * coord
        c2r2[2] = 1.0
        c2r2[3] = r2
        crm = np.empty((4, C), np.float32)        # rhs rows [x, y, r2, 1]
        crm[0:2] = coord[:, cols]
        crm[2] = r2[cols]
        crm[3] = 1.0
        embT0 = np.zeros((N, 2 * F), np.float32)
        embT0[:, 0:F0] = gi[b].T
        embT0[:, F:F + F0] = 1.0 / 64.0
        in_maps.append(dict(
            c2_all=c2r2.astype(ml_dtypes.bfloat16),
            coords_my=crm.astype(ml_dtypes.bfloat16),
            tabs=tabs,
            embT0_ones=embT0.astype(ml_dtypes.bfloat16),
            embT0_8=embT0.astype(ml_dtypes.float8_e4m3),
            emb0_loc=np.ascontiguousarray(gi[b][:, cols]).astype(ml_dtypes.bfloat16),
            wT0=wT0.astype(ml_dtypes.bfloat16),
            wT12=wT12.astype(ml_dtypes.bfloat16),
            bcat=np.ascontiguousarray(bcat).astype(ml_dtypes.bfloat16),
            ident=ident,
        ))
    return in_maps


def kernel(global_input, sigmas, w_lin0, b_lin0, w_nlin0, b_nlin0,
           w_lin, b_lin, w_nlin, b_nlin, fcl_w, fcl_b):
    if "nc" not in _CACHE:
        _CACHE["nc"] = _build_program()
    nc = _CACHE["nc"]
    in_maps = _host_inputs(global_input, sigmas, w_lin0, b_lin0, w_nlin0,
                           b_nlin0, w_lin, b_lin, w_nlin, b_nlin)
    res = run_bass_kernel_spmd(nc, in_maps, core_ids=list(range(8)))
    pooled = np.empty((B, F), np.float64)
    for b in range(B):
        s0 = res.results[2 * b]["pooled_partial"].reshape(F)
        s1 = res.results[2 * b + 1]["pooled_partial"].reshape(F)
        pooled[b] = (s0.astype(np.float64) + s1.astype(np.float64)) / N
    mu = pooled.mean(axis=1, keepdims=True)
    var = pooled.var(axis=1, keepdims=True)
    normed = (pooled - mu) / np.sqrt(var + EPS_IN)
    logits = normed @ np.asarray(fcl_w, np.float64).T + np.asarray(fcl_b, np.float64)
    out = 1.0 / (1.0 + np.exp(-logits[:, 0]))
    return out.astype(np.float32)
